# revision 1
# baseline (speedup 1.0000x reference)
"""DeepseekV2 MoE layer on 8 Trainium2 NeuronCores.

Strategy (expert-parallel, per the sharding hint):
  - Router gate + grouped top-k computed on host (0.03% of module FLOPs);
    it determines the dispatch, which IS the input sharding.
  - 16 routed experts paired big-count-with-small-count onto 8 cores
    (2 experts per core, token lists gathered host-side, padded to a
    shared per-slot capacity so all cores run one SPMD program).
  - Shared-expert MLP is data-parallel over tokens: each core runs
    T/8 = 512 tokens through the full shared MLP.
  - All matmuls in bf16 (fp32 PE matmul is 2x slower), f32 PSUM
    accumulation, f32 outputs.
  - Device computes outputs token-on-free-dim (transposed); host
    transposes/combines during unshard.
"""

import sys

sys.path.insert(0, "/opt/trn_rl_repo")

import copy

import ml_dtypes
import numpy as np

import concourse.bass as bass
import concourse.mybir as mybir
import concourse.tile as tile
from concourse.bass_utils import run_bass_kernel_spmd

DT = mybir.dt
BF16 = ml_dtypes.bfloat16

T, D, E, I = 4096, 2048, 16, 1024
TOP_K, N_GROUP, TOPK_GROUP = 4, 4, 2
ROUTED_SCALE = 2.5
SHARED_I = 2048
N_CORES = 8
P = 128
NCHUNK = 512  # token chunk (matmul moving free dim)


# ---------------------------------------------------------------- wait split
def _split_excess_waits(nc, limit=1):
    """This walrus build rejects >1 sync-wait command per instruction.
    Move excess waits onto fresh same-engine NOPs inserted just before."""
    template = bass.Bass(target_bir_lowering=False).sync.nop(nofuse=True).ins
    ctr = 0
    for bb in nc.main_func.blocks:
        out = []
        changed = False
        for ins in bb.instructions:
            si = ins.sync_info
            if si is not None and si.on_wait and len(si.on_wait) > limit:
                waits = list(si.on_wait)
                for w in waits[:-limit]:
                    ctr += 1
                    nop = copy.deepcopy(template)
                    nop.name = f"I-wsplit-{ctr}"
                    nop.engine = ins.engine
                    nop.bass_nofuse = True
                    nop.sync_info = mybir.SyncInfo(on_wait=[w], on_update=[])
                    nc.register_instruction(nop, overwrite=True)
                    out.append(nop)
                ins.sync_info = mybir.SyncInfo(
                    on_wait=waits[-limit:], on_update=list(si.on_update)
                )
                changed = True
            out.append(ins)
        if changed:
            bb.instructions = out
    return ctr


# ---------------------------------------------------------------- routing
def _gate_logits(x, gate_w):
    # Match the reference's jax-f32 CPU matmul as closely as possible.
    try:
        import jax
        import jax.numpy as jnp

        cpu = jax.devices("cpu")[0]
        with jax.default_device(cpu):
            return np.asarray(jnp.matmul(jnp.asarray(x), jnp.asarray(gate_w)))
    except Exception:
        return (x @ gate_w).astype(np.float32)


def _route(x, gate_w, e_bias):
    logits = _gate_logits(x, gate_w)  # [T, E] f32
    scores = (1.0 / (1.0 + np.exp(-logits))).astype(np.float32)
    sfc = scores + e_bias[None, :]
    grp = sfc.reshape(T, N_GROUP, E // N_GROUP)
    group_scores = np.sort(grp, axis=-1)[:, :, -2:].sum(-1)  # [T, G]
    group_idx = np.argsort(-group_scores, axis=-1, kind="stable")[:, :TOPK_GROUP]
    group_mask = np.zeros((T, N_GROUP), bool)
    group_mask[np.arange(T)[:, None], group_idx] = True
    expert_mask = np.repeat(group_mask, E // N_GROUP, axis=1)
    masked = np.where(expert_mask, sfc, -np.inf)
    topk_idx = np.argsort(-masked, axis=-1, kind="stable")[:, :TOP_K]  # [T, 4]
    topk_w = np.take_along_axis(scores, topk_idx, axis=1)
    topk_w = topk_w / topk_w.sum(axis=1, keepdims=True)
    return topk_idx.astype(np.int64), topk_w.astype(np.float32)


# ---------------------------------------------------------------- program
_PROGRAM_CACHE = {}


def _emit_expert(nc, tc, pools, xt_h, w1_h, w2_h, wr_h, y_h, C, twoI, apply_wr):
    n_d = D // P  # 16 contraction chunks over D
    n_i = twoI // P  # gate_up output chunks
    n_h = n_i // 2  # h chunks (= I/128)
    chunks = [(o, min(NCHUNK, C - o)) for o in range(0, C, NCHUNK)]

    (xt_pool, w1_pool, w2_pool, g_pool, h_pool, y_pool, wr_pool, sg_pool,
     ps_gu, ps_dn) = pools

    # whole-expert X^T resident tile: [p, k(d-chunk), tok]; split the load
    # per d-chunk so the first matmuls start as soon as chunk 0 lands
    xt_t = xt_pool.tile([P, n_d, C], DT.bfloat16, name="xt")
    src = xt_h[:, :].rearrange("(k p) t -> p k t", p=P)
    for d in range(n_d):
        nc.sync.dma_start(xt_t[:, d, :], src[:, d, :])

    wr_t = None
    if apply_wr:
        wr_t = wr_pool.tile([P, C], DT.float32, name="wr")
        nc.sync.dma_start(wr_t[:], wr_h[:, :])

    g_tiles = {}
    h_tiles = {}
    for i in range(n_i):
        w1s = w1_pool.tile([P, n_d, P], DT.bfloat16, name="w1s")
        nc.sync.dma_start(w1s[:], w1_h[i])
        for ci, (off, sz) in enumerate(chunks):
            ps = ps_gu.tile([P, NCHUNK], DT.float32, name="psg")
            for d in range(n_d):
                nc.tensor.matmul(
                    ps[:, :sz],
                    w1s[:, d, :],
                    xt_t[:, d, off : off + sz],
                    start=(d == 0),
                    stop=(d == n_d - 1),
                )
            if i < n_h:
                sg = sg_pool.tile([P, NCHUNK], DT.float32, name="sg")
                nc.scalar.activation(
                    sg[:, :sz], ps[:, :sz], mybir.ActivationFunctionType.Sigmoid
                )
                gt = g_pool.tile([P, NCHUNK], DT.float32, name="gt")
                nc.vector.tensor_mul(gt[:, :sz], ps[:, :sz], sg[:, :sz])
                g_tiles[(i, ci)] = gt
            else:
                ht = h_pool.tile([P, NCHUNK], DT.bfloat16, name="ht")
                nc.vector.tensor_mul(
                    ht[:, :sz], ps[:, :sz], g_tiles[(i - n_h, ci)][:, :sz]
                )
                h_tiles[(i - n_h, ci)] = ht

    for d2 in range(D // P):
        w2s = w2_pool.tile([P, n_h, P], DT.bfloat16, name="w2s")
        nc.sync.dma_start(w2s[:], w2_h[d2])
        for ci, (off, sz) in enumerate(chunks):
            ps = ps_dn.tile([P, NCHUNK], DT.float32, name="psd")
            for hh in range(n_h):
                nc.tensor.matmul(
                    ps[:, :sz],
                    w2s[:, hh, :],
                    h_tiles[(hh, ci)][:, :sz],
                    start=(hh == 0),
                    stop=(hh == n_h - 1),
                )
            ys = y_pool.tile([P, NCHUNK], DT.float32, name="ys")
            if apply_wr:
                nc.vector.tensor_mul(ys[:, :sz], ps[:, :sz], wr_t[:, off : off + sz])
            else:
                nc.scalar.copy(ys[:, :sz], ps[:, :sz])
            nc.sync.dma_start(y_h[d2 * P : (d2 + 1) * P, off : off + sz], ys[:, :sz])


def _build_program(C1, C2):
    key = (C1, C2)
    if key in _PROGRAM_CACHE:
        return _PROGRAM_CACHE[key]

    nc = bass.Bass(target_bir_lowering=False)
    TS = T // N_CORES  # shared tokens per core

    xt1 = nc.dram_tensor("xt1", [D, C1], DT.bfloat16, kind="ExternalInput")
    xt2 = nc.dram_tensor("xt2", [D, C2], DT.bfloat16, kind="ExternalInput")
    xts = nc.dram_tensor("xts", [D, TS], DT.bfloat16, kind="ExternalInput")
    w1a = nc.dram_tensor("w1a", [2 * I // P, P, D // P, P], DT.bfloat16, kind="ExternalInput")
    w2a = nc.dram_tensor("w2a", [D // P, P, I // P, P], DT.bfloat16, kind="ExternalInput")
    w1b = nc.dram_tensor("w1b", [2 * I // P, P, D // P, P], DT.bfloat16, kind="ExternalInput")
    w2b = nc.dram_tensor("w2b", [D // P, P, I // P, P], DT.bfloat16, kind="ExternalInput")
    ws1 = nc.dram_tensor("ws1", [2 * SHARED_I // P, P, D // P, P], DT.bfloat16, kind="ExternalInput")
    ws2 = nc.dram_tensor("ws2", [D // P, P, SHARED_I // P, P], DT.bfloat16, kind="ExternalInput")
    wr1 = nc.dram_tensor("wr1", [P, C1], DT.float32, kind="ExternalInput")
    wr2 = nc.dram_tensor("wr2", [P, C2], DT.float32, kind="ExternalInput")
    y1 = nc.dram_tensor("y1", [D, C1], DT.float32, kind="ExternalOutput")
    y2 = nc.dram_tensor("y2", [D, C2], DT.float32, kind="ExternalOutput")
    ys = nc.dram_tensor("ys", [D, TS], DT.float32, kind="ExternalOutput")

    with tile.TileContext(nc) as tc:
        with (
            tc.tile_pool(name="xt", bufs=1) as xt_pool,
            tc.tile_pool(name="w1p", bufs=3) as w1_pool,
            tc.tile_pool(name="w2p", bufs=3) as w2_pool,
            tc.tile_pool(name="gp", bufs=24) as g_pool,
            tc.tile_pool(name="hp", bufs=24) as h_pool,
            tc.tile_pool(name="yp", bufs=4) as y_pool,
            tc.tile_pool(name="wrp", bufs=2) as wr_pool,
            tc.tile_pool(name="sgp", bufs=3) as sg_pool,
            tc.tile_pool(name="psgu", bufs=4, space="PSUM") as ps_gu,
            tc.tile_pool(name="psdn", bufs=4, space="PSUM") as ps_dn,
        ):
            pools = (xt_pool, w1_pool, w2_pool, g_pool, h_pool, y_pool, wr_pool,
                     sg_pool, ps_gu, ps_dn)
            _emit_expert(nc, tc, pools, xt1, w1a, w2a, wr1, y1, C1, 2 * I, True)
            _emit_expert(nc, tc, pools, xt2, w1b, w2b, wr2, y2, C2, 2 * I, True)
            _emit_expert(nc, tc, pools, xts, ws1, ws2, None, ys, T // N_CORES, 2 * SHARED_I, False)

    _split_excess_waits(nc, limit=1)
    _PROGRAM_CACHE[key] = nc
    return nc


# ---------------------------------------------------------------- packing
def _pack_w1(w, twoI):  # w [D, twoI] f32 -> [twoI/P, P, D/P, P] bf16
    n_d, n_i = D // P, twoI // P
    return np.ascontiguousarray(
        w.astype(BF16).reshape(n_d, P, n_i, P).transpose(2, 1, 0, 3)
    )


def _pack_w2(w, I_):  # w [I_, D] f32 -> [D/P, P, I_/P, P] bf16
    n_h, n_d2 = I_ // P, D // P
    return np.ascontiguousarray(
        w.astype(BF16).reshape(n_h, P, n_d2, P).transpose(2, 1, 0, 3)
    )


def _cap(n):
    # exact capacity (matmul free dim handles any size <= 512 per chunk);
    # keep a small floor so shapes stay sane for degenerate routings
    return max(P, int(n))


# ---------------------------------------------------------------- kernel
def _prepare(hidden_states, gate_w, e_bias, w_gate_up, w_down, ws_gate_up, ws_down):
    x = np.asarray(hidden_states, dtype=np.float32)
    topk_idx, topk_w = _route(x, np.asarray(gate_w), np.asarray(e_bias))

    # dispatch: token lists per expert, sorted-stable by expert id
    flat_e = topk_idx.ravel()
    order = np.argsort(flat_e, kind="stable")
    pair_tok = order // TOP_K
    pair_w = (topk_w.ravel()[order] * ROUTED_SCALE).astype(np.float32)
    counts = np.bincount(flat_e, minlength=E)
    starts = np.zeros(E + 1, np.int64)
    np.cumsum(counts, out=starts[1:])

    # expert -> core assignment: pair largest with smallest
    by_count = np.argsort(-counts, kind="stable")
    slotA = by_count[:N_CORES]
    slotB = by_count[E - 1 : N_CORES - 1 : -1]  # reversed smallest half
    C1 = _cap(counts[slotA].max())
    C2 = _cap(counts[slotB].max())

    nc = _build_program(C1, C2)

    x_bf = x.astype(BF16)
    xT_bf = np.ascontiguousarray(x_bf.T)  # [D, T]

    ws1_p = _pack_w1(np.asarray(ws_gate_up), 2 * SHARED_I)
    ws2_p = _pack_w2(np.asarray(ws_down), SHARED_I)
    w_gate_up = np.asarray(w_gate_up)
    w_down = np.asarray(w_down)

    TS = T // N_CORES
    in_maps = []
    core_info = []
    for c in range(N_CORES):
        eA, eB = int(slotA[c]), int(slotB[c])
        m = {}
        info = []
        for slot, e_id, C, xt_name, wr_name in (
            (0, eA, C1, "xt1", "wr1"),
            (1, eB, C2, "xt2", "wr2"),
        ):
            idx = pair_tok[starts[e_id] : starts[e_id + 1]]
            w = pair_w[starts[e_id] : starts[e_id + 1]]
            n_e = len(idx)
            idx_pad = np.zeros(C, np.int64)
            idx_pad[:n_e] = idx
            w_pad = np.zeros(C, np.float32)
            w_pad[:n_e] = w
            m[xt_name] = xT_bf[:, idx_pad]
            m[wr_name] = np.ascontiguousarray(
                np.broadcast_to(w_pad, (P, C))
            )
            info.append((idx, n_e))
        m["xts"] = np.ascontiguousarray(xT_bf[:, c * TS : (c + 1) * TS])
        m["w1a"] = _pack_w1(w_gate_up[eA], 2 * I)
        m["w2a"] = _pack_w2(w_down[eA], I)
        m["w1b"] = _pack_w1(w_gate_up[eB], 2 * I)
        m["w2b"] = _pack_w2(w_down[eB], I)
        m["ws1"] = ws1_p
        m["ws2"] = ws2_p
        in_maps.append(m)
        core_info.append(info)
    return nc, in_maps, core_info


def _combine(res_results, core_info):
    TS = T // N_CORES
    out = np.zeros((T, D), np.float32)
    for c in range(N_CORES):
        (idxA, nA), (idxB, nB) = core_info[c]
        out[idxA] += res_results[c]["y1"][:, :nA].T
        out[idxB] += res_results[c]["y2"][:, :nB].T
        out[c * TS : (c + 1) * TS] += res_results[c]["ys"].T
    return out


def kernel(hidden_states, gate_w, e_bias, w_gate_up, w_down, ws_gate_up, ws_down):
    nc, in_maps, core_info = _prepare(
        hidden_states, gate_w, e_bias, w_gate_up, w_down, ws_gate_up, ws_down
    )
    res = run_bass_kernel_spmd(nc, in_maps, list(range(N_CORES)))
    return _combine(res.results, core_info)



# revision 4
# speedup vs baseline: 1.3387x; 1.3387x over previous
"""DeepseekV2 MoE layer on 8 Trainium2 NeuronCores.

Strategy (expert-parallel, per the sharding hint):
  - Router gate + grouped top-k computed on host (0.03% of module FLOPs);
    it determines the dispatch, which IS the input sharding.
  - 16 routed experts paired big-count-with-small-count onto 8 cores
    (2 experts per core, token lists gathered host-side, padded to a
    shared per-slot capacity so all cores run one SPMD program).
  - Shared-expert MLP is data-parallel over tokens: each core runs
    T/8 = 512 tokens through the full shared MLP.
  - All matmuls run as fp8(e4m3) DoubleRow with a 3-term hi/lo
    decomposition:  W·x ~= Whi·xhi + Wlo·xhi + Whi·xlo  accumulated in
    one f32 PSUM chain.  This costs 0.75x the bf16 PE cycles (DoubleRow
    contracts K=256 per instruction at 0.5 cycles/row) while being MORE
    accurate than bf16 (the dropped Wlo·xlo term is ~0.06%).
  - x and W hi/lo splits are computed host-side at fixed power-of-2
    scales (SX=32, SW=2048); the intermediate h = silu(g)*u is split
    on device (Sigmoid on ACT + fused silu-mul / quantize / residual
    on DVE), h at scale SH=8.
  - Outputs leave the device as f16 (routing weight * 2.5 / PSUM-scale
    folded into a per-token f32 vector for the routed experts).
"""

import sys

sys.path.insert(0, "/opt/trn_rl_repo")

import copy

import ml_dtypes
import numpy as np

import concourse.bass as bass
import concourse.mybir as mybir
import concourse.tile as tile
from concourse.bass_utils import run_bass_kernel_spmd

DT = mybir.dt
E4 = ml_dtypes.float8_e4m3

T, D, E, I = 4096, 2048, 16, 1024
TOP_K, N_GROUP, TOPK_GROUP = 4, 4, 2
ROUTED_SCALE = 2.5
SHARED_I = 2048
N_CORES = 8
P = 128
NCHUNK = 512

# fp8 scales (all powers of two; residuals stored unscaled in e4m3)
SX = 32.0
SW = 2048.0
SH = 8.0
PS1 = SX * SW            # L1 PSUM scale (65536)
QH = SH / PS1            # h quant factor (2**-13)
PS2 = SH * SW            # L2 PSUM scale (16384)

DR = mybir.MatmulPerfMode.DoubleRow
MULT = mybir.AluOpType.mult
SUBTRACT = mybir.AluOpType.subtract


# ---------------------------------------------------------------- wait split
def _split_excess_waits(nc, limit=1):
    """This walrus build rejects >1 sync-wait command per instruction.
    Move excess waits onto fresh same-engine NOPs inserted just before."""
    template = bass.Bass(target_bir_lowering=False).sync.nop(nofuse=True).ins
    ctr = 0
    for bb in nc.main_func.blocks:
        out = []
        changed = False
        for ins in bb.instructions:
            si = ins.sync_info
            if si is not None and si.on_wait and len(si.on_wait) > limit:
                waits = list(si.on_wait)
                for w in waits[:-limit]:
                    ctr += 1
                    nop = copy.deepcopy(template)
                    nop.name = f"I-wsplit-{ctr}"
                    nop.engine = ins.engine
                    nop.bass_nofuse = True
                    nop.sync_info = mybir.SyncInfo(on_wait=[w], on_update=[])
                    nc.register_instruction(nop, overwrite=True)
                    out.append(nop)
                ins.sync_info = mybir.SyncInfo(
                    on_wait=waits[-limit:], on_update=list(si.on_update)
                )
                changed = True
            out.append(ins)
        if changed:
            bb.instructions = out
    return ctr


# ---------------------------------------------------------------- routing
def _gate_logits(x, gate_w):
    # Match the reference's jax-f32 CPU matmul as closely as possible.
    try:
        import jax
        import jax.numpy as jnp

        cpu = jax.devices("cpu")[0]
        with jax.default_device(cpu):
            return np.asarray(jnp.matmul(jnp.asarray(x), jnp.asarray(gate_w)))
    except Exception:
        return (x @ gate_w).astype(np.float32)


def _route(x, gate_w, e_bias):
    logits = _gate_logits(x, gate_w)  # [T, E] f32
    scores = (1.0 / (1.0 + np.exp(-logits))).astype(np.float32)
    sfc = scores + e_bias[None, :]
    grp = sfc.reshape(T, N_GROUP, E // N_GROUP)
    group_scores = np.sort(grp, axis=-1)[:, :, -2:].sum(-1)  # [T, G]
    group_idx = np.argsort(-group_scores, axis=-1, kind="stable")[:, :TOPK_GROUP]
    group_mask = np.zeros((T, N_GROUP), bool)
    group_mask[np.arange(T)[:, None], group_idx] = True
    expert_mask = np.repeat(group_mask, E // N_GROUP, axis=1)
    masked = np.where(expert_mask, sfc, -np.inf)
    topk_idx = np.argsort(-masked, axis=-1, kind="stable")[:, :TOP_K]  # [T, 4]
    topk_w = np.take_along_axis(scores, topk_idx, axis=1)
    topk_w = topk_w / topk_w.sum(axis=1, keepdims=True)
    return topk_idx.astype(np.int64), topk_w.astype(np.float32)


# ---------------------------------------------------------------- packing
def _pair8(a, s):
    """f32 array -> (hi, lo) e4m3 at scale s; lo holds the unscaled residual."""
    a = np.asarray(a, np.float32) * s
    hi = a.astype(E4)
    lo = (a - hi.astype(np.float32)).astype(E4)
    return hi, lo


def _pack_stat(w8):
    """[K, M] e4m3 -> [M//P, P, K//256, 2, P] DoubleRow stationary layout."""
    K, M = w8.shape
    return np.ascontiguousarray(
        w8.reshape(K // 256, 2, P, M // P, P).transpose(3, 2, 0, 1, 4)
    )


def _pack_mov(x8rows):
    """[C, K] e4m3 (row per token) -> [P, K//256, 2, C] DoubleRow moving."""
    C, K = x8rows.shape
    return np.ascontiguousarray(
        x8rows.reshape(C, K // 256, 2, P).transpose(3, 1, 2, 0)
    )


def _chunks(C):
    return [(o, min(NCHUNK, C - o)) for o in range(0, C, NCHUNK)]


# ---------------------------------------------------------------- program
_PROGRAM_CACHE = {}


def _emit_l1(nc, pools, xth, xtl, w1h, w1l, h8hi, h8lo, C, n_h):
    """L1 (gate_up) + silu-mul + h quantize for one expert slot.
    w1h/w1l: dram [2*n_h, P, 8, 2, P].  h8 tiles: [P, n_h//2, 2, C]."""
    w1_pool, ps_g, ps_u, sg_pool, silu_pool, ht_pool = pools
    for hh in range(n_h):
        wgh = w1_pool.tile([P, 8, 2, P], DT.float8e4, name="wgh", tag="w1", bufs=10)
        nc.gpsimd.dma_start(wgh, w1h[hh])
        wgl = w1_pool.tile([P, 8, 2, P], DT.float8e4, name="wgl", tag="w1", bufs=10)
        nc.gpsimd.dma_start(wgl, w1l[hh])
        wuh = w1_pool.tile([P, 8, 2, P], DT.float8e4, name="wuh", tag="w1", bufs=10)
        nc.gpsimd.dma_start(wuh, w1h[n_h + hh])
        wul = w1_pool.tile([P, 8, 2, P], DT.float8e4, name="wul", tag="w1", bufs=10)
        nc.gpsimd.dma_start(wul, w1l[n_h + hh])
        for off, sz in _chunks(C):
            psg = ps_g.tile([P, NCHUNK], DT.float32, name="psg", tag="psg")
            k = 0
            for wt, xt in ((wgh, xth), (wgl, xth), (wgh, xtl)):
                for j in range(8):
                    nc.tensor.matmul(
                        psg[:, :sz], wt[:, j], xt[:, j, :, off : off + sz],
                        start=(k == 0), stop=(k == 23), perf_mode=DR,
                    )
                    k += 1
            psu = ps_u.tile([P, NCHUNK], DT.float32, name="psu", tag="psu")
            k = 0
            for wt, xt in ((wuh, xth), (wul, xth), (wuh, xtl)):
                for j in range(8):
                    nc.tensor.matmul(
                        psu[:, :sz], wt[:, j], xt[:, j, :, off : off + sz],
                        start=(k == 0), stop=(k == 23), perf_mode=DR,
                    )
                    k += 1
            sg = sg_pool.tile([P, NCHUNK], DT.float32, name="sg", tag="sg")
            nc.scalar.activation(
                sg[:, :sz], psg[:, :sz],
                mybir.ActivationFunctionType.Sigmoid, scale=1.0 / PS1,
            )
            silu = silu_pool.tile([P, NCHUNK], DT.float32, name="silu", tag="silu")
            nc.vector.scalar_tensor_tensor(
                silu[:, :sz], psg[:, :sz], 1.0 / PS1, sg[:, :sz], MULT, MULT
            )
            ht = ht_pool.tile([P, NCHUNK], DT.float32, name="ht", tag="ht")
            nc.vector.tensor_mul(ht[:, :sz], silu[:, :sz], psu[:, :sz])
            hv = h8hi[:, hh // 2, hh % 2, off : off + sz]
            nc.scalar.mul(hv, ht[:, :sz], QH)
            nc.vector.scalar_tensor_tensor(
                h8lo[:, hh // 2, hh % 2, off : off + sz],
                ht[:, :sz], QH, hv, MULT, SUBTRACT,
            )


def _emit_l2(nc, pools, h8hi, h8lo, w2h, w2l, y, wr, C, n_j2):
    """L2 (down proj) + routing-weight scale + f16 output DMA.
    w2h/w2l: dram [16, P, n_j2, 2, P].  wr: SBUF [P, C] f32 or None."""
    w2_pool, ps_d, y_pool = pools
    for d2 in range(16):
        wdh = w2_pool.tile([P, n_j2, 2, P], DT.float8e4, name="wdh", tag="w2", bufs=8)
        nc.gpsimd.dma_start(wdh, w2h[d2])
        wdl = w2_pool.tile([P, n_j2, 2, P], DT.float8e4, name="wdl", tag="w2", bufs=8)
        nc.gpsimd.dma_start(wdl, w2l[d2])
        for off, sz in _chunks(C):
            psd = ps_d.tile([P, NCHUNK], DT.float32, name="psd", tag="psd")
            k = 0
            for wt, hm in ((wdh, h8hi), (wdl, h8hi), (wdh, h8lo)):
                for j in range(n_j2):
                    nc.tensor.matmul(
                        psd[:, :sz], wt[:, j], hm[:, j, :, off : off + sz],
                        start=(k == 0), stop=(k == 3 * n_j2 - 1), perf_mode=DR,
                    )
                    k += 1
            yt = y_pool.tile([P, NCHUNK], DT.float16, name="yt", tag="y")
            if wr is not None:
                nc.vector.tensor_mul(yt[:, :sz], psd[:, :sz], wr[:, off : off + sz])
            else:
                nc.scalar.mul(yt[:, :sz], psd[:, :sz], 1.0 / PS2)
            nc.sync.dma_start(y[d2 * P : (d2 + 1) * P, off : off + sz], yt[:, :sz])


def _build_program(C1, C2):
    key = (C1, C2)
    if key in _PROGRAM_CACHE:
        return _PROGRAM_CACHE[key]

    nc = bass.Bass(target_bir_lowering=False)
    TS = T // N_CORES

    def din(name, shape, dt=DT.float8e4):
        return nc.dram_tensor(name, shape, dt, kind="ExternalInput")

    xt1h = din("xt1h", [P, 8, 2, C1]); xt1l = din("xt1l", [P, 8, 2, C1])
    xt2h = din("xt2h", [P, 8, 2, C2]); xt2l = din("xt2l", [P, 8, 2, C2])
    xtsh = din("xtsh", [P, 8, 2, TS]); xtsl = din("xtsl", [P, 8, 2, TS])
    w1ah = din("w1ah", [16, P, 8, 2, P]); w1al = din("w1al", [16, P, 8, 2, P])
    w2ah = din("w2ah", [16, P, 4, 2, P]); w2al = din("w2al", [16, P, 4, 2, P])
    w1bh = din("w1bh", [16, P, 8, 2, P]); w1bl = din("w1bl", [16, P, 8, 2, P])
    w2bh = din("w2bh", [16, P, 4, 2, P]); w2bl = din("w2bl", [16, P, 4, 2, P])
    ws1h = din("ws1h", [32, P, 8, 2, P]); ws1l = din("ws1l", [32, P, 8, 2, P])
    ws2h = din("ws2h", [16, P, 8, 2, P]); ws2l = din("ws2l", [16, P, 8, 2, P])
    wr1 = din("wr1", [P, C1], DT.float32)
    wr2 = din("wr2", [P, C2], DT.float32)
    y1 = nc.dram_tensor("y1", [D, C1], DT.float16, kind="ExternalOutput")
    y2 = nc.dram_tensor("y2", [D, C2], DT.float16, kind="ExternalOutput")
    ys = nc.dram_tensor("ys", [D, TS], DT.float16, kind="ExternalOutput")

    with tile.TileContext(nc) as tc:
        with (
            tc.tile_pool(name="xt", bufs=1) as xt_pool,
            tc.tile_pool(name="h8", bufs=1) as h8_pool,
            tc.tile_pool(name="w1p", bufs=10) as w1_pool,
            tc.tile_pool(name="w2p", bufs=8) as w2_pool,
            tc.tile_pool(name="sgp", bufs=3) as sg_pool,
            tc.tile_pool(name="silup", bufs=3) as silu_pool,
            tc.tile_pool(name="htp", bufs=3) as ht_pool,
            tc.tile_pool(name="yp", bufs=4) as y_pool,
            tc.tile_pool(name="wrp", bufs=1) as wr_pool,
            tc.tile_pool(name="psg", bufs=2, space="PSUM") as ps_g,
            tc.tile_pool(name="psu", bufs=2, space="PSUM") as ps_u,
            tc.tile_pool(name="psd", bufs=3, space="PSUM") as ps_d,
        ):
            l1_pools = (w1_pool, ps_g, ps_u, sg_pool, silu_pool, ht_pool)
            l2_pools = (w2_pool, ps_d, y_pool)

            def xt_tiles(dramh, draml, C, tagh, tagl):
                th = xt_pool.tile([P, 8, 2, C], DT.float8e4, name="xth", tag=tagh, bufs=1)
                tl = xt_pool.tile([P, 8, 2, C], DT.float8e4, name="xtl", tag=tagl, bufs=1)
                for j in range(8):
                    nc.sync.dma_start(th[:, j], dramh[:, j])
                for j in range(8):
                    nc.sync.dma_start(tl[:, j], draml[:, j])
                return th, tl

            def h8_tiles(C, n_j2, tagh, tagl):
                hh = h8_pool.tile([P, n_j2, 2, C], DT.float8e4, name="h8h", tag=tagh, bufs=1)
                hl = h8_pool.tile([P, n_j2, 2, C], DT.float8e4, name="h8l", tag=tagl, bufs=1)
                return hh, hl

            # initial loads: expert A tokens, routing weights, expert B tokens
            xtah, xtal = xt_tiles(xt1h, xt1l, C1, "xah", "xal")
            wra = wr_pool.tile([P, C1], DT.float32, name="wra", tag="wra", bufs=1)
            nc.sync.dma_start(wra, wr1[:, :])
            wrb = wr_pool.tile([P, C2], DT.float32, name="wrb", tag="wrb", bufs=1)
            nc.sync.dma_start(wrb, wr2[:, :])
            xtbh, xtbl = xt_tiles(xt2h, xt2l, C2, "xbh", "xbl")

            h8ah, h8al = h8_tiles(C1, 4, "hah", "hal")
            h8bh, h8bl = h8_tiles(C2, 4, "hbh", "hbl")

            _emit_l1(nc, l1_pools, xtah, xtal, w1ah, w1al, h8ah, h8al, C1, 8)
            _emit_l1(nc, l1_pools, xtbh, xtbl, w1bh, w1bl, h8bh, h8bl, C2, 8)

            # shared-expert tokens reuse expert A's xt slots (A L1 is done
            # by the time these DMAs land); emitted before L2(A) so the
            # transfers overlap with L2(A)/L1(S) compute.
            xtsh_t, xtsl_t = xt_tiles(xtsh, xtsl, TS, "xah", "xal")

            _emit_l2(nc, l2_pools, h8ah, h8al, w2ah, w2al, y1, wra, C1, 4)

            h8sh, h8sl = h8_tiles(TS, 8, "hah", "hal")
            _emit_l1(nc, l1_pools, xtsh_t, xtsl_t, ws1h, ws1l, h8sh, h8sl, TS, 16)

            _emit_l2(nc, l2_pools, h8bh, h8bl, w2bh, w2bl, y2, wrb, C2, 4)
            _emit_l2(nc, l2_pools, h8sh, h8sl, ws2h, ws2l, ys, None, TS, 8)

    _split_excess_waits(nc, limit=1)
    _PROGRAM_CACHE[key] = nc
    return nc


def _cap(n):
    # Round capacity up to a multiple of 16: the PE requires >=4-byte-aligned
    # access-pattern base offsets, and the [P, n_j2, 2, C] fp8 layouts slice
    # at j*2*C / hh*C byte offsets (misaligned C crashes the exec unit).
    return max(P, (int(n) + 15) & ~15)


# ---------------------------------------------------------------- kernel
def _prepare(hidden_states, gate_w, e_bias, w_gate_up, w_down, ws_gate_up, ws_down):
    x = np.asarray(hidden_states, dtype=np.float32)
    topk_idx, topk_w = _route(x, np.asarray(gate_w), np.asarray(e_bias))

    # dispatch: token lists per expert, sorted-stable by expert id
    flat_e = topk_idx.ravel()
    order = np.argsort(flat_e, kind="stable")
    pair_tok = order // TOP_K
    pair_w = (topk_w.ravel()[order] * (ROUTED_SCALE / PS2)).astype(np.float32)
    counts = np.bincount(flat_e, minlength=E)
    starts = np.zeros(E + 1, np.int64)
    np.cumsum(counts, out=starts[1:])

    # expert -> core assignment: pair largest with smallest
    by_count = np.argsort(-counts, kind="stable")
    slotA = by_count[:N_CORES]
    slotB = by_count[E - 1 : N_CORES - 1 : -1]  # reversed smallest half
    C1 = _cap(counts[slotA].max())
    C2 = _cap(counts[slotB].max())

    nc = _build_program(C1, C2)

    # global fp8 splits
    xhi, xlo = _pair8(x, SX)                       # [T, D] e4m3
    w_gate_up = np.asarray(w_gate_up)
    w_down = np.asarray(w_down)
    w1p = [tuple(_pack_stat(q) for q in _pair8(w_gate_up[e], SW)) for e in range(E)]
    w2p = [tuple(_pack_stat(q) for q in _pair8(w_down[e], SW)) for e in range(E)]
    ws1h_, ws1l_ = (_pack_stat(q) for q in _pair8(np.asarray(ws_gate_up), SW))
    ws2h_, ws2l_ = (_pack_stat(q) for q in _pair8(np.asarray(ws_down), SW))

    TS = T // N_CORES
    in_maps = []
    core_info = []
    for c in range(N_CORES):
        eA, eB = int(slotA[c]), int(slotB[c])
        m = {}
        info = []
        for e_id, C, xh_name, xl_name, wr_name in (
            (eA, C1, "xt1h", "xt1l", "wr1"),
            (eB, C2, "xt2h", "xt2l", "wr2"),
        ):
            idx = pair_tok[starts[e_id] : starts[e_id + 1]]
            w = pair_w[starts[e_id] : starts[e_id + 1]]
            n_e = len(idx)
            idx_pad = np.zeros(C, np.int64)
            idx_pad[:n_e] = idx
            w_pad = np.zeros(C, np.float32)
            w_pad[:n_e] = w
            m[xh_name] = _pack_mov(xhi[idx_pad])
            m[xl_name] = _pack_mov(xlo[idx_pad])
            m[wr_name] = np.ascontiguousarray(np.broadcast_to(w_pad, (P, C)))
            info.append((idx, n_e))
        sl = slice(c * TS, (c + 1) * TS)
        m["xtsh"] = _pack_mov(xhi[sl])
        m["xtsl"] = _pack_mov(xlo[sl])
        m["w1ah"], m["w1al"] = w1p[eA]
        m["w2ah"], m["w2al"] = w2p[eA]
        m["w1bh"], m["w1bl"] = w1p[eB]
        m["w2bh"], m["w2bl"] = w2p[eB]
        m["ws1h"], m["ws1l"] = ws1h_, ws1l_
        m["ws2h"], m["ws2l"] = ws2h_, ws2l_
        in_maps.append(m)
        core_info.append(info)
    return nc, in_maps, core_info


def _combine(res_results, core_info):
    TS = T // N_CORES
    out = np.zeros((T, D), np.float32)
    for c in range(N_CORES):
        (idxA, nA), (idxB, nB) = core_info[c]
        out[idxA] += res_results[c]["y1"][:, :nA].T.astype(np.float32)
        out[idxB] += res_results[c]["y2"][:, :nB].T.astype(np.float32)
        out[c * TS : (c + 1) * TS] += res_results[c]["ys"].T.astype(np.float32)
    return out


def kernel(hidden_states, gate_w, e_bias, w_gate_up, w_down, ws_gate_up, ws_down):
    nc, in_maps, core_info = _prepare(
        hidden_states, gate_w, e_bias, w_gate_up, w_down, ws_gate_up, ws_down
    )
    res = run_bass_kernel_spmd(nc, in_maps, list(range(N_CORES)))
    return _combine(res.results, core_info)


# revision 22
# speedup vs baseline: 1.3921x; 1.0399x over previous
"""DeepseekV2 MoE layer on 8 Trainium2 NeuronCores.

Strategy (expert-parallel, per the sharding hint):
  - Router gate + grouped top-k computed on host (0.03% of module FLOPs);
    it determines the dispatch, which IS the input sharding.
  - Routed experts are packed into 3 SPMD capacity slots per core
    (capacities solved at dispatch time by a small bucket-packing DP;
    large experts split across cores), so per-core padded work is close
    to the 2048-token ideal.  Shared-expert MLP is data-parallel over
    tokens: each core runs T/8 = 512 tokens through the full shared MLP.
  - All matmuls run as fp8(e4m3) DoubleRow with a 3-term hi/lo
    decomposition:  W·x ~= Whi·xhi + Wlo·xhi + Whi·xlo  accumulated in
    one f32 PSUM chain.  This costs 0.75x the bf16 PE cycles (DoubleRow
    contracts K=256 per instruction at 0.5 cycles/row) while being MORE
    accurate than bf16 (the dropped Wlo·xlo term is ~0.06%).
  - x and W hi/lo splits are computed host-side at fixed power-of-2
    scales (SX=32, SW=2048); the intermediate h = silu(g)*u is split
    on device (Sigmoid on ACT + fused silu-mul / quantize / residual
    on DVE), h at scale SH=8.
  - Outputs leave the device as f16 (routing weight * 2.5 / PSUM-scale
    folded into a per-token f32 vector for the routed experts).
  - Emission interleaves the small slot-C expert work into the larger
    L2/L1 windows of slots A/S so its weight stream prefetches cleanly.
"""

import sys

sys.path.insert(0, "/opt/trn_rl_repo")

import copy
import itertools
from functools import lru_cache

import ml_dtypes
import numpy as np

import concourse.bass as bass
import concourse.mybir as mybir
import concourse.tile as tile
from concourse.bass_utils import run_bass_kernel_spmd

DT = mybir.dt
E4 = ml_dtypes.float8_e4m3

T, D, E, I = 4096, 2048, 16, 1024
TOP_K, N_GROUP, TOPK_GROUP = 4, 4, 2
ROUTED_SCALE = 2.5
SHARED_I = 2048
N_CORES = 8
P = 128
NCHUNK = 512

# fp8 scales (all powers of two; residuals stored unscaled in e4m3)
SX = 32.0
SW = 2048.0
SH = 8.0
PS1 = SX * SW            # L1 PSUM scale (65536)
QH = SH / PS1            # h quant factor (2**-13)
PS2 = SH * SW            # L2 PSUM scale (16384)

DR = mybir.MatmulPerfMode.DoubleRow
MULT = mybir.AluOpType.mult
SUBTRACT = mybir.AluOpType.subtract

# tunables (swept via TimelineSim; safe defaults)
CONFIG = {
    "lo_q": "sync",      # queue for lo weight tiles: "gpsimd" | "sync" | "scalar"
    "hi_q": "gpsimd",    # queue for hi weight tiles
    "xt_q": "scalar",    # queue for xt/wr loads
    "y_q": "sync",       # queue for y output DMAs
    "hhi_eng": "scalar", # engine for h-hi quant: "scalar" | "vector"
    "psg": 2, "psu": 2, "psd": 4,
    "w1_bufs": 8, "w2_bufs": 6,
    "weave_a": 2, "weave_s": 1,
    "act_bufs": 3, "y_bufs": 4,
    "order": "afirst",   # "afirst" | "sfirst" (shared expert first)
}


def _q(nc, name):
    return getattr(nc, CONFIG[name])


# ---------------------------------------------------------------- wait split
def _split_excess_waits(nc, limit=1):
    """This walrus build rejects >1 sync-wait command per instruction.
    Move excess waits onto fresh same-engine NOPs inserted just before."""
    template = bass.Bass(target_bir_lowering=False).sync.nop(nofuse=True).ins
    ctr = 0
    for bb in nc.main_func.blocks:
        out = []
        changed = False
        for ins in bb.instructions:
            si = ins.sync_info
            if si is not None and si.on_wait and len(si.on_wait) > limit:
                waits = list(si.on_wait)
                for w in waits[:-limit]:
                    ctr += 1
                    nop = copy.deepcopy(template)
                    nop.name = f"I-wsplit-{ctr}"
                    nop.engine = ins.engine
                    nop.bass_nofuse = True
                    nop.sync_info = mybir.SyncInfo(on_wait=[w], on_update=[])
                    nc.register_instruction(nop, overwrite=True)
                    out.append(nop)
                ins.sync_info = mybir.SyncInfo(
                    on_wait=waits[-limit:], on_update=list(si.on_update)
                )
                changed = True
            out.append(ins)
        if changed:
            bb.instructions = out
    return ctr


# ---------------------------------------------------------------- routing
def _gate_logits(x, gate_w):
    # Match the reference's jax-f32 CPU matmul as closely as possible.
    try:
        import jax
        import jax.numpy as jnp

        cpu = jax.devices("cpu")[0]
        with jax.default_device(cpu):
            return np.asarray(jnp.matmul(jnp.asarray(x), jnp.asarray(gate_w)))
    except Exception:
        return (x @ gate_w).astype(np.float32)


def _route(x, gate_w, e_bias):
    logits = _gate_logits(x, gate_w)  # [T, E] f32
    scores = (1.0 / (1.0 + np.exp(-logits))).astype(np.float32)
    sfc = scores + e_bias[None, :]
    grp = sfc.reshape(T, N_GROUP, E // N_GROUP)
    group_scores = np.sort(grp, axis=-1)[:, :, -2:].sum(-1)  # [T, G]
    group_idx = np.argsort(-group_scores, axis=-1, kind="stable")[:, :TOPK_GROUP]
    group_mask = np.zeros((T, N_GROUP), bool)
    group_mask[np.arange(T)[:, None], group_idx] = True
    expert_mask = np.repeat(group_mask, E // N_GROUP, axis=1)
    masked = np.where(expert_mask, sfc, -np.inf)
    topk_idx = np.argsort(-masked, axis=-1, kind="stable")[:, :TOP_K]  # [T, 4]
    topk_w = np.take_along_axis(scores, topk_idx, axis=1)
    topk_w = topk_w / topk_w.sum(axis=1, keepdims=True)
    return topk_idx.astype(np.int64), topk_w.astype(np.float32)


# ---------------------------------------------------------------- packing
def _r16(v):
    # The PE requires >=4-byte-aligned access-pattern base offsets; the
    # [P, n_j2, 2, C] fp8 layouts slice at j*2*C byte offsets, so C must
    # be a multiple of >=2 -- use 16 for comfortable alignment everywhere.
    return max(P, (int(v) + 15) & ~15)


def _solve_slots(counts):
    """Pick 3 slot capacities + an expert->buckets assignment minimizing
    C1+C2+C3.  Buckets: 8 per slot (one per core); an expert may span
    several buckets (capacities summing >= its count)."""
    cands = set()
    for c in counts:
        cands.add(_r16(c))
        cands.add(_r16((c + 1) // 2))
        cands.add(_r16((c + 2) // 3))
    for a in counts:
        for b in counts:
            if a > b:
                cands.add(_r16(a - b))
    cands = sorted(x for x in cands if P <= x <= _r16(max(counts)))
    base = _r16(sorted(counts, reverse=True)[0]) + _r16(sorted(counts, reverse=True)[8])

    order = sorted(range(len(counts)), key=lambda e: -counts[e])
    cs = [counts[e] for e in order]

    def assignment(caps):
        profiles = []
        for r in range(1, 4):
            for combo in itertools.combinations_with_replacement(range(3), r):
                profiles.append((combo, sum(caps[s] for s in combo)))

        @lru_cache(maxsize=None)
        def rec(i, used):
            if i == len(cs):
                return ()
            for combo, cap in profiles:
                if cap < cs[i]:
                    continue
                u2 = list(used)
                ok = True
                for s in combo:
                    u2[s] += 1
                    if u2[s] > N_CORES:
                        ok = False
                        break
                if ok:
                    r = rec(i + 1, tuple(u2))
                    if r is not None:
                        return ((order[i], combo),) + r
            return None

        return rec(0, (0, 0, 0))

    trips = []
    for c1 in cands:
        for c2 in cands:
            if c2 > c1:
                continue
            for c3 in cands:
                if c3 > c2:
                    continue
                S = c1 + c2 + c3
                if S < base + P:
                    trips.append((S, (c1, c2, c3)))
    trips.sort()
    for S, caps in trips:
        asg = assignment(caps)
        if asg is not None:
            return caps, asg
    # fallback: big-with-small 2-slot + empty third slot
    caps = (base - _r16(sorted(counts, reverse=True)[8]),
            _r16(sorted(counts, reverse=True)[8]), P)
    by = sorted(range(len(counts)), key=lambda e: -counts[e])
    asg = tuple((e, (0,)) for e in by[:8]) + tuple((e, (1,)) for e in by[8:])
    return caps, asg


def _pair8(a, s):
    """f32 array -> (hi, lo) e4m3 at scale s; lo holds the unscaled residual."""
    a = np.asarray(a, np.float32) * s
    hi = a.astype(E4)
    lo = (a - hi.astype(np.float32)).astype(E4)
    return hi, lo


def _pack_stat(w8):
    """[K, M] e4m3 -> [M//P, P, K//256, 2, P] DoubleRow stationary layout."""
    K, M = w8.shape
    return np.ascontiguousarray(
        w8.reshape(K // 256, 2, P, M // P, P).transpose(3, 2, 0, 1, 4)
    )


def _pack_mov(x8rows):
    """[C, K] e4m3 (row per token) -> [P, K//256, 2, C] DoubleRow moving."""
    C, K = x8rows.shape
    return np.ascontiguousarray(
        x8rows.reshape(C, K // 256, 2, P).transpose(3, 1, 2, 0)
    )


def _chunks(C):
    return [(o, min(NCHUNK, C - o)) for o in range(0, C, NCHUNK)]


def _chunks_rampup(C):
    """Chunk split that fronts small chunks so the first matmul chains can
    start before most of the xt tokens have landed (slot A startup)."""
    out = []
    tail = C % NCHUNK
    if tail:
        out.append((C - tail, tail))
    if C >= NCHUNK:
        out.extend([(0, 256), (256, 256)])
        out.extend((o, NCHUNK) for o in range(NCHUNK, C - tail, NCHUNK))
    return out


# ---------------------------------------------------------------- program
_PROGRAM_CACHE = {}


def _l1_units(nc, pools, xth, xtl, w1h, w1l, h8hi, h8lo, C, n_h, chunks=None):
    """Return per-h-group emission closures for L1 (gate_up + h quant)."""
    w1_pool, ps_g, ps_u, sg_pool, silu_pool, ht_pool = pools
    if chunks is None:
        chunks = _chunks(C)

    def unit(hh):
        def emit():
            wgh = w1_pool.tile([P, 8, 2, P], DT.float8e4, name="wgh", tag="w1", bufs=CONFIG["w1_bufs"])
            _q(nc, "hi_q").dma_start(wgh, w1h[hh])
            wgl = w1_pool.tile([P, 8, 2, P], DT.float8e4, name="wgl", tag="w1", bufs=CONFIG["w1_bufs"])
            _q(nc, "lo_q").dma_start(wgl, w1l[hh])
            wuh = w1_pool.tile([P, 8, 2, P], DT.float8e4, name="wuh", tag="w1", bufs=CONFIG["w1_bufs"])
            _q(nc, "hi_q").dma_start(wuh, w1h[n_h + hh])
            wul = w1_pool.tile([P, 8, 2, P], DT.float8e4, name="wul", tag="w1", bufs=CONFIG["w1_bufs"])
            _q(nc, "lo_q").dma_start(wul, w1l[n_h + hh])
            for off, sz in chunks:
                psg = ps_g.tile([P, NCHUNK], DT.float32, name="psg", tag="psg")
                k = 0
                for wt, xt in ((wgh, xth), (wgl, xth), (wgh, xtl)):
                    for j in range(8):
                        nc.tensor.matmul(
                            psg[:, :sz], wt[:, j], xt[:, j, :, off : off + sz],
                            start=(k == 0), stop=(k == 23), perf_mode=DR,
                        )
                        k += 1
                psu = ps_u.tile([P, NCHUNK], DT.float32, name="psu", tag="psu")
                k = 0
                for wt, xt in ((wuh, xth), (wul, xth), (wuh, xtl)):
                    for j in range(8):
                        nc.tensor.matmul(
                            psu[:, :sz], wt[:, j], xt[:, j, :, off : off + sz],
                            start=(k == 0), stop=(k == 23), perf_mode=DR,
                        )
                        k += 1
                sg = sg_pool.tile([P, NCHUNK], DT.float32, name="sg", tag="sg")
                nc.scalar.activation(
                    sg[:, :sz], psg[:, :sz],
                    mybir.ActivationFunctionType.Sigmoid, scale=1.0 / PS1,
                )
                silu = silu_pool.tile([P, NCHUNK], DT.float32, name="silu", tag="silu")
                nc.vector.scalar_tensor_tensor(
                    silu[:, :sz], psg[:, :sz], 1.0 / PS1, sg[:, :sz], MULT, MULT
                )
                ht = ht_pool.tile([P, NCHUNK], DT.float32, name="ht", tag="ht")
                nc.vector.tensor_mul(ht[:, :sz], silu[:, :sz], psu[:, :sz])
                hv = h8hi[:, hh // 2, hh % 2, off : off + sz]
                if CONFIG["hhi_eng"] == "scalar":
                    nc.scalar.mul(hv, ht[:, :sz], QH)
                else:
                    nc.vector.tensor_scalar_mul(hv, ht[:, :sz], QH)
                nc.vector.scalar_tensor_tensor(
                    h8lo[:, hh // 2, hh % 2, off : off + sz],
                    ht[:, :sz], QH, hv, MULT, SUBTRACT,
                )
        return emit

    return [unit(hh) for hh in range(n_h)]


def _l2_units(nc, pools, h8hi, h8lo, w2h, w2l, y, wr, C, n_j2):
    """Return per-out-chunk emission closures for L2 (down proj + output)."""
    w2_pool, ps_d, y_pool = pools

    def unit(d2):
        def emit():
            wdh = w2_pool.tile([P, n_j2, 2, P], DT.float8e4, name="wdh", tag="w2", bufs=CONFIG["w2_bufs"])
            _q(nc, "hi_q").dma_start(wdh, w2h[d2])
            wdl = w2_pool.tile([P, n_j2, 2, P], DT.float8e4, name="wdl", tag="w2", bufs=CONFIG["w2_bufs"])
            _q(nc, "lo_q").dma_start(wdl, w2l[d2])
            for off, sz in _chunks(C):
                psd = ps_d.tile([P, NCHUNK], DT.float32, name="psd", tag="psd")
                k = 0
                for wt, hm in ((wdh, h8hi), (wdl, h8hi), (wdh, h8lo)):
                    for j in range(n_j2):
                        nc.tensor.matmul(
                            psd[:, :sz], wt[:, j], hm[:, j, :, off : off + sz],
                            start=(k == 0), stop=(k == 3 * n_j2 - 1), perf_mode=DR,
                        )
                        k += 1
                yt = y_pool.tile([P, NCHUNK], DT.float16, name="yt", tag="y")
                if wr is not None:
                    nc.vector.tensor_mul(yt[:, :sz], psd[:, :sz], wr[:, off : off + sz])
                else:
                    nc.scalar.mul(yt[:, :sz], psd[:, :sz], 1.0 / PS2)
                _q(nc, "y_q").dma_start(y[d2 * P : (d2 + 1) * P, off : off + sz], yt[:, :sz])
        return emit

    return [unit(d2) for d2 in range(16)]


def _weave(big, small, ratio):
    """Interleave emission unit lists: `ratio` big units per small unit."""
    out = []
    bi = si = 0
    while bi < len(big) or si < len(small):
        for _ in range(ratio):
            if bi < len(big):
                out.append(big[bi]); bi += 1
        if si < len(small):
            out.append(small[si]); si += 1
    return out


def _build_program(C1, C2, C3):
    key = (C1, C2, C3)
    if key in _PROGRAM_CACHE:
        return _PROGRAM_CACHE[key]

    nc = bass.Bass(target_bir_lowering=False)
    TS = T // N_CORES

    def din(name, shape, dt=DT.float8e4):
        return nc.dram_tensor(name, shape, dt, kind="ExternalInput")

    xt1h = din("xt1h", [P, 8, 2, C1]); xt1l = din("xt1l", [P, 8, 2, C1])
    xt2h = din("xt2h", [P, 8, 2, C2]); xt2l = din("xt2l", [P, 8, 2, C2])
    xt3h = din("xt3h", [P, 8, 2, C3]); xt3l = din("xt3l", [P, 8, 2, C3])
    xtsh = din("xtsh", [P, 8, 2, TS]); xtsl = din("xtsl", [P, 8, 2, TS])
    w1ah = din("w1ah", [16, P, 8, 2, P]); w1al = din("w1al", [16, P, 8, 2, P])
    w2ah = din("w2ah", [16, P, 4, 2, P]); w2al = din("w2al", [16, P, 4, 2, P])
    w1bh = din("w1bh", [16, P, 8, 2, P]); w1bl = din("w1bl", [16, P, 8, 2, P])
    w2bh = din("w2bh", [16, P, 4, 2, P]); w2bl = din("w2bl", [16, P, 4, 2, P])
    w1ch = din("w1ch", [16, P, 8, 2, P]); w1cl = din("w1cl", [16, P, 8, 2, P])
    w2ch = din("w2ch", [16, P, 4, 2, P]); w2cl = din("w2cl", [16, P, 4, 2, P])
    ws1h = din("ws1h", [32, P, 8, 2, P]); ws1l = din("ws1l", [32, P, 8, 2, P])
    ws2h = din("ws2h", [16, P, 8, 2, P]); ws2l = din("ws2l", [16, P, 8, 2, P])
    wr1 = din("wr1", [P, C1], DT.float32)
    wr2 = din("wr2", [P, C2], DT.float32)
    wr3 = din("wr3", [P, C3], DT.float32)
    y1 = nc.dram_tensor("y1", [D, C1], DT.float16, kind="ExternalOutput")
    y2 = nc.dram_tensor("y2", [D, C2], DT.float16, kind="ExternalOutput")
    y3 = nc.dram_tensor("y3", [D, C3], DT.float16, kind="ExternalOutput")
    ys = nc.dram_tensor("ys", [D, TS], DT.float16, kind="ExternalOutput")

    with tile.TileContext(nc) as tc:
        with (
            tc.tile_pool(name="xt", bufs=1) as xt_pool,
            tc.tile_pool(name="h8", bufs=1) as h8_pool,
            tc.tile_pool(name="w1p", bufs=CONFIG["w1_bufs"]) as w1_pool,
            tc.tile_pool(name="w2p", bufs=CONFIG["w2_bufs"]) as w2_pool,
            tc.tile_pool(name="sgp", bufs=CONFIG["act_bufs"]) as sg_pool,
            tc.tile_pool(name="silup", bufs=CONFIG["act_bufs"]) as silu_pool,
            tc.tile_pool(name="htp", bufs=CONFIG["act_bufs"]) as ht_pool,
            tc.tile_pool(name="yp", bufs=CONFIG["y_bufs"]) as y_pool,
            tc.tile_pool(name="wrp", bufs=1) as wr_pool,
            tc.tile_pool(name="psg", bufs=CONFIG["psg"], space="PSUM") as ps_g,
            tc.tile_pool(name="psu", bufs=CONFIG["psu"], space="PSUM") as ps_u,
            tc.tile_pool(name="psd", bufs=CONFIG["psd"], space="PSUM") as ps_d,
        ):
            l1_pools = (w1_pool, ps_g, ps_u, sg_pool, silu_pool, ht_pool)
            l2_pools = (w2_pool, ps_d, y_pool)

            def xt_tiles(dramh, draml, C, tagh, tagl, c0_last=False):
                # per-chunk column slices (single DMA each)
                th = xt_pool.tile([P, 8, 2, C], DT.float8e4, name="xth", tag=tagh, bufs=1)
                tl = xt_pool.tile([P, 8, 2, C], DT.float8e4, name="xtl", tag=tagl, bufs=1)
                order = _chunks(C)
                if c0_last and len(order) > 1:
                    # chunk 0 lands LAST: the first PE chain then waits once
                    # for the whole load instead of stuttering per chunk
                    # (each stutter would reset the PE p-state ramp)
                    order = order[1:] + order[:1]
                for off, sz in order:
                    _q(nc, "xt_q").dma_start(tl[:, :, :, off : off + sz],
                                             draml[:, :, :, off : off + sz])
                    _q(nc, "xt_q").dma_start(th[:, :, :, off : off + sz],
                                             dramh[:, :, :, off : off + sz])
                return th, tl

            def h8_tiles(C, n_j2, tagh, tagl):
                hh = h8_pool.tile([P, n_j2, 2, C], DT.float8e4, name="h8h", tag=tagh, bufs=1)
                hl = h8_pool.tile([P, n_j2, 2, C], DT.float8e4, name="h8l", tag=tagl, bufs=1)
                return hh, hl

            def wr_tile(dram, C, tag):
                t = wr_pool.tile([P, C], DT.float32, name="wr", tag=tag, bufs=1)
                _q(nc, "xt_q").dma_start(t, dram[:, :])
                return t

            h8bh, h8bl = h8_tiles(C2, 4, "hbh", "hbl")
            h8ch, h8cl = h8_tiles(C3, 4, "hch", "hcl")

            if CONFIG["order"] == "afirst":
                # A-L1 first; B/C token loads emitted mid-A-L1 so they don't
                # steal early DMA bandwidth
                xtah, xtal = xt_tiles(xt1h, xt1l, C1, "xah", "xal", c0_last=CONFIG.get("c0_last", False))
                wra = wr_tile(wr1, C1, "wra")
                h8ah, h8al = h8_tiles(C1, 4, "hah", "hal")
                a_l1 = _l1_units(nc, l1_pools, xtah, xtal, w1ah, w1al, h8ah, h8al, C1, 8)
                for u in a_l1[:4]:
                    u()
                xtbh, xtbl = xt_tiles(xt2h, xt2l, C2, "xbh", "xbl")
                wrb = wr_tile(wr2, C2, "wrb")
                for u in a_l1[4:6]:
                    u()
                xtch, xtcl = xt_tiles(xt3h, xt3l, C3, "xch", "xcl")
                wrc = wr_tile(wr3, C3, "wrc")
                for u in a_l1[6:]:
                    u()

                b_l1 = _l1_units(nc, l1_pools, xtbh, xtbl, w1bh, w1bl, h8bh, h8bl, C2, 8)
                c_l1 = _l1_units(nc, l1_pools, xtch, xtcl, w1ch, w1cl, h8ch, h8cl, C3, 8)
                a_l2 = _l2_units(nc, l2_pools, h8ah, h8al, w2ah, w2al, y1, wra, C1, 4)
                b_l2 = _l2_units(nc, l2_pools, h8bh, h8bl, w2bh, w2bl, y2, wrb, C2, 4)
                c_l2 = _l2_units(nc, l2_pools, h8ch, h8cl, w2ch, w2cl, y3, wrc, C3, 4)

                for u in b_l1:
                    u()

                # shared-expert tokens reuse expert A's xt slots (A L1 done)
                xtsh_t, xtsl_t = xt_tiles(xtsh, xtsl, TS, "xah", "xal")

                # weave slot-C L1 into A-L2's window
                for u in _weave(a_l2, c_l1, CONFIG["weave_a"]):
                    u()

                h8sh, h8sl = h8_tiles(TS, 8, "hah", "hal")
                s_l1 = _l1_units(nc, l1_pools, xtsh_t, xtsl_t, ws1h, ws1l, h8sh, h8sl, TS, 16)
                s_l2 = _l2_units(nc, l2_pools, h8sh, h8sl, ws2h, ws2l, ys, None, TS, 8)

                # weave slot-C L2 into S-L1's window
                for u in _weave(s_l1, c_l2, CONFIG["weave_s"]):
                    u()
                for u in b_l2:
                    u()
                for u in s_l2:
                    u()
            else:
                # shared expert first: its token load is small, so the PE
                # ramps in fast while A/B tokens prefetch during S-L1
                xtsh_t, xtsl_t = xt_tiles(xtsh, xtsl, TS, "xsh", "xsl")
                h8sh, h8sl = h8_tiles(TS, 8, "hsh", "hsl")
                s_l1 = _l1_units(nc, l1_pools, xtsh_t, xtsl_t, ws1h, ws1l, h8sh, h8sl, TS, 16)
                for u in s_l1[:3]:
                    u()
                xtah, xtal = xt_tiles(xt1h, xt1l, C1, "xah", "xal")
                wra = wr_tile(wr1, C1, "wra")
                for u in s_l1[3:8]:
                    u()
                xtbh, xtbl = xt_tiles(xt2h, xt2l, C2, "xbh", "xbl")
                wrb = wr_tile(wr2, C2, "wrb")
                for u in s_l1[8:12]:
                    u()
                xtch, xtcl = xt_tiles(xt3h, xt3l, C3, "xch", "xcl")
                wrc = wr_tile(wr3, C3, "wrc")
                for u in s_l1[12:]:
                    u()

                h8ah, h8al = h8_tiles(C1, 4, "hah", "hal")
                a_l1 = _l1_units(nc, l1_pools, xtah, xtal, w1ah, w1al, h8ah, h8al, C1, 8)
                b_l1 = _l1_units(nc, l1_pools, xtbh, xtbl, w1bh, w1bl, h8bh, h8bl, C2, 8)
                c_l1 = _l1_units(nc, l1_pools, xtch, xtcl, w1ch, w1cl, h8ch, h8cl, C3, 8)
                a_l2 = _l2_units(nc, l2_pools, h8ah, h8al, w2ah, w2al, y1, wra, C1, 4)
                b_l2 = _l2_units(nc, l2_pools, h8bh, h8bl, w2bh, w2bl, y2, wrb, C2, 4)
                c_l2 = _l2_units(nc, l2_pools, h8ch, h8cl, w2ch, w2cl, y3, wrc, C3, 4)
                s_l2 = _l2_units(nc, l2_pools, h8sh, h8sl, ws2h, ws2l, ys, None, TS, 8)

                for u in a_l1:
                    u()
                for u in b_l1:
                    u()
                for u in _weave(a_l2, c_l1, CONFIG["weave_a"]):
                    u()
                for u in _weave(s_l2, c_l2, CONFIG["weave_s"]):
                    u()
                for u in b_l2:
                    u()

    _split_excess_waits(nc, limit=1)
    _PROGRAM_CACHE[key] = nc
    return nc


# ---------------------------------------------------------------- kernel
def _prepare(hidden_states, gate_w, e_bias, w_gate_up, w_down, ws_gate_up, ws_down):
    x = np.asarray(hidden_states, dtype=np.float32)
    topk_idx, topk_w = _route(x, np.asarray(gate_w), np.asarray(e_bias))

    # dispatch: token lists per expert, sorted-stable by expert id
    flat_e = topk_idx.ravel()
    order = np.argsort(flat_e, kind="stable")
    pair_tok = order // TOP_K
    pair_w = (topk_w.ravel()[order] * (ROUTED_SCALE / PS2)).astype(np.float32)
    counts = np.bincount(flat_e, minlength=E)
    starts = np.zeros(E + 1, np.int64)
    np.cumsum(counts, out=starts[1:])

    caps, asg = _solve_slots([int(c) for c in counts])
    C1, C2, C3 = caps

    # slot buckets: per slot, list of (expert, tok_idx_array, weights) per core
    buckets = {0: [], 1: [], 2: []}
    for e_id, combo in asg:
        toks = pair_tok[starts[e_id] : starts[e_id + 1]]
        ws = pair_w[starts[e_id] : starts[e_id + 1]]
        pos = 0
        # fill largest-capacity buckets first
        for s in sorted(combo, key=lambda s: -caps[s]):
            n = min(len(toks) - pos, caps[s])
            buckets[s].append((e_id, toks[pos : pos + n], ws[pos : pos + n]))
            pos += n
        assert pos == len(toks), f"expert {e_id} unpacked: {pos}/{len(toks)}"
    for s in range(3):
        while len(buckets[s]) < N_CORES:
            buckets[s].append((0, np.zeros(0, np.int64), np.zeros(0, np.float32)))
        assert len(buckets[s]) == N_CORES

    # avoid placing two buckets of one expert on the same core in the same...
    # (different slots is fine; same slot cannot happen twice on one core
    #  because we just assign bucket i of each slot to core i, and the DP
    #  never assigns more than 8 buckets per slot)

    nc = _build_program(C1, C2, C3)

    # global fp8 splits
    xhi, xlo = _pair8(x, SX)                       # [T, D] e4m3
    w_gate_up = np.asarray(w_gate_up)
    w_down = np.asarray(w_down)
    w1p = [tuple(_pack_stat(q) for q in _pair8(w_gate_up[e], SW)) for e in range(E)]
    w2p = [tuple(_pack_stat(q) for q in _pair8(w_down[e], SW)) for e in range(E)]
    ws1h_, ws1l_ = (_pack_stat(q) for q in _pair8(np.asarray(ws_gate_up), SW))
    ws2h_, ws2l_ = (_pack_stat(q) for q in _pair8(np.asarray(ws_down), SW))

    TS = T // N_CORES
    in_maps = []
    core_info = []
    for c in range(N_CORES):
        m = {}
        info = []
        for s, C, xh_name, xl_name, wr_name, wkey in (
            (0, C1, "xt1h", "xt1l", "wr1", "a"),
            (1, C2, "xt2h", "xt2l", "wr2", "b"),
            (2, C3, "xt3h", "xt3l", "wr3", "c"),
        ):
            e_id, idx, w = buckets[s][c]
            n_e = len(idx)
            idx_pad = np.zeros(C, np.int64)
            idx_pad[:n_e] = idx
            w_pad = np.zeros(C, np.float32)
            w_pad[:n_e] = w
            m[xh_name] = _pack_mov(xhi[idx_pad])
            m[xl_name] = _pack_mov(xlo[idx_pad])
            m[wr_name] = np.ascontiguousarray(np.broadcast_to(w_pad, (P, C)))
            m[f"w1{wkey}h"], m[f"w1{wkey}l"] = w1p[e_id]
            m[f"w2{wkey}h"], m[f"w2{wkey}l"] = w2p[e_id]
            info.append((idx, n_e))
        sl = slice(c * TS, (c + 1) * TS)
        m["xtsh"] = _pack_mov(xhi[sl])
        m["xtsl"] = _pack_mov(xlo[sl])
        m["ws1h"], m["ws1l"] = ws1h_, ws1l_
        m["ws2h"], m["ws2l"] = ws2h_, ws2l_
        in_maps.append(m)
        core_info.append(info)
    return nc, in_maps, core_info


def _combine(res_results, core_info):
    TS = T // N_CORES
    out = np.zeros((T, D), np.float32)
    for c in range(N_CORES):
        for (idx, n_e), y_nm in zip(core_info[c], ("y1", "y2", "y3")):
            if n_e:
                out[idx] += res_results[c][y_nm][:, :n_e].T.astype(np.float32)
        out[c * TS : (c + 1) * TS] += res_results[c]["ys"].T.astype(np.float32)
    return out


def kernel(hidden_states, gate_w, e_bias, w_gate_up, w_down, ws_gate_up, ws_down):
    nc, in_maps, core_info = _prepare(
        hidden_states, gate_w, e_bias, w_gate_up, w_down, ws_gate_up, ws_down
    )
    res = run_bass_kernel_spmd(nc, in_maps, list(range(N_CORES)))
    return _combine(res.results, core_info)
